# revision 21
# baseline (speedup 1.0000x reference)
"""Trainium2 Bass kernel for a bilinear decoder:

    u = z_user[row]; m = z_movie[col]                      # [E, 64] gathers
    logits[e, r] = u_e^T Q_r m_e                           # [E, 5]
    out = log_softmax(logits, axis=-1)

Strategy (v2): 2-D cell sharding + batched SWDGE dma_gather.

The SWDGE batched-gather ucode (InstDMAGatherAnt, mlp GPSIMD library,
compiled via Bacc which auto-inserts the library load) gathers up to
~1920 rows per instruction but its int16 indices address at most 32768
table rows.  So edges are sharded host-side into a 4x4 grid of
(user-chunk, movie-chunk) cells of 25000 rows each -- two cells per
core, sharing one z_user slice -- and every gather addresses a <=25000
row slice with int16 indices.  Tables are converted host-side to bf16
padded to 256 B rows (the gather's granularity).

Per 896-edge tile: two 896-index dma_gathers (round-robin over 4 SWDGE
queues; 57 descriptors each, two in flight per 128-descriptor ring), 7
per-group PE transposes (bf16 PSUM), one ACT copy to SBUF, 7 matmuls
(u^T chunks x Qflat bf16, outputs at 2 KB bank-aligned PSUM offsets --
matmul outputs that straddle a PSUM bank boundary silently corrupt),
ACT f32->bf16 W copy, bf16 DVE multiply+reduce.  log_softmax is
batched over 4-tile super-tiles so the ACT engine's Exp/Ln activation
table reloads (~1.3 us each) amortize; that batching took 1.22 ms ->
0.80 ms.

Empirical limits (measured here): >1024 indices per gather overflows
the 128-descriptor/lane SWDGE ring and wedges the core (the ring size
is an NRT-side constant; neither dynamic_dma_scratch_size nor walrus'
--dynamic-dma-scratch-size-per-partition moves it).  The Pool engine
serializes desc-gen at ~994 ns + ~1.2 ns/idx per instruction -- with
288 gathers/core that is ~620 us and is this kernel's wall.

Measured on 8 axon trn2 cores: rel err 3.4e-3 (bf16 tables/compute;
harness gate 2e-2), HW exec time 0.80 ms vs 2.76 ms baseline (3.45x).
"""

import os
import numpy as np
import ml_dtypes

import concourse.bacc as bacc
import concourse.bass as bass
import concourse.mybir as mybir
import concourse.tile as tile
from concourse.masks import make_identity
from concourse.bass_utils import run_bass_kernel_spmd

F32 = mybir.dt.float32
BF16 = mybir.dt.bfloat16
I16 = mybir.dt.int16

N_ROWS = 100000
E_TOTAL = 1000000
D = 64
R = 5
N_CORES = 8

CHUNK = 25000            # table rows per cell slice (fits int16 indices)
GRID = 4                 # 4x4 grid of (user-chunk, movie-chunk) cells
CAP = 64512              # padded edges per cell; 63 tiles of 1024
TILE_E = 1024            # edges per gather instruction (65 descs/ring)
NT_CELL = CAP // TILE_E  # 63
GROUPS = TILE_E // 128   # 8 edge groups per tile
N_QUEUES = 4
SUPER = 21             # tiles per softmax flush (amortizes ACT table reloads)


def build_nc():
    nc = bacc.Bacc("TRN2", num_swdge_queues=N_QUEUES)
    zu = nc.dram_tensor("zu", [CHUNK, 2 * D], BF16, kind="ExternalInput")
    zm0 = nc.dram_tensor("zm0", [CHUNK, 2 * D], BF16, kind="ExternalInput")
    zm1 = nc.dram_tensor("zm1", [CHUNK, 2 * D], BF16, kind="ExternalInput")
    # int16 idx streams, 16-partition wrapped and replicated to 128 partitions
    idxu = [nc.dram_tensor(f"idxu{s}", [128, CAP // 16], I16, kind="ExternalInput")
            for s in range(2)]
    idxm = [nc.dram_tensor(f"idxm{s}", [128, CAP // 16], I16, kind="ExternalInput")
            for s in range(2)]
    # block-diag [[Qflat, 0], [0, Qflat]] with Qflat[k, (r, l)] = Q[r, k, l]
    qbd = nc.dram_tensor("qbd", [128, 2 * R * D], BF16, kind="ExternalInput")
    out = nc.dram_tensor("out", [2 * CAP, R], F32, kind="ExternalOutput")

    gather_i = 0
    with tile.TileContext(nc) as tc:
        with (
            tc.tile_pool(name="const", bufs=1) as cpool,
            tc.tile_pool(name="io", bufs=8) as iopool,
            tc.tile_pool(name="work", bufs=3) as wpool,
            tc.tile_pool(name="ps_t", bufs=2, space="PSUM") as tpsum,
            tc.tile_pool(name="ps_w", bufs=3, space="PSUM") as wpsum,
        ):
            ident = cpool.tile([128, 128], BF16)
            make_identity(nc, ident[:])
            qsb = cpool.tile([128, 2 * R * D], BF16)
            nc.sync.dma_start(out=qsb[:], in_=qbd[:])
            idxu_sb = [cpool.tile([128, CAP // 16], I16, name=f"idxu_sb{i}") for i in range(2)]
            idxm_sb = [cpool.tile([128, CAP // 16], I16, name=f"idxm_sb{i}") for i in range(2)]
            for s in range(2):
                nc.sync.dma_start(out=idxu_sb[s][:], in_=idxu[s][:])
                nc.sync.dma_start(out=idxm_sb[s][:], in_=idxm[s][:])

            for s in range(2):
                zm = zm0 if s == 0 else zm1
                for t0 in range(0, NT_CELL, SUPER):
                    logits = wpool.tile([128, SUPER * GROUPS * R], F32, tag="lg")
                    for j in range(SUPER):
                        gather_i = emit_tile(
                            nc, s, t0 + j, j, zu, zm, idxu_sb[s], idxm_sb[s],
                            qsb, ident, logits, iopool, wpool, tpsum, wpsum,
                            gather_i)
                    emit_softmax(nc, s, t0, logits, out, wpool)
    return nc


def emit_tile(nc, s, t, j, zu, zm, idxu_sb, idxm_sb, qsb, ident, logits,
              iopool, wpool, tpsum, wpsum, gather_i):
    c0 = t * (TILE_E // 16)
    c1 = (t + 1) * (TILE_E // 16)

    # ---- gathers: [128, g, 128] bf16; idx i -> partition i%128, group i//128
    ut = iopool.tile([128, GROUPS, 128], BF16, tag="ut")
    mt = iopool.tile([128, GROUPS, 128], BF16, tag="mt")
    nc.gpsimd.dma_gather(
        out_ap=ut[:], in_ap=zu[:], idxs_ap=idxu_sb[:, c0:c1],
        num_idxs=TILE_E, num_idxs_reg=TILE_E, elem_size=2 * D,
        queue_num=gather_i % N_QUEUES)
    gather_i += 1
    nc.gpsimd.dma_gather(
        out_ap=mt[:], in_ap=zm[:], idxs_ap=idxm_sb[:, c0:c1],
        num_idxs=TILE_E, num_idxs_reg=TILE_E, elem_size=2 * D,
        queue_num=gather_i % N_QUEUES)
    gather_i += 1

    # ---- transposes: u^T per group, [64 l-partitions, 128 edge cols]
    n_pairs = (GROUPS + 1) // 2  # 4 (last pair is a single group)
    tps = tpsum.tile([128, GROUPS * 128], BF16, tag="tps")
    for g in range(GROUPS):
        nc.tensor.transpose(
            out=tps[:D, g * 128: g * 128 + 128],
            in_=ut[:, g, :D],
            identity=ident[:],
        )
    utq = wpool.tile([128, GROUPS * 128], BF16, tag="utq")
    nc.scalar.copy(out=utq[:D], in_=tps[:D])

    for h in range(n_pairs):
        g0 = 2 * h
        ng = min(2, GROUPS - g0)
        # W chunks at 512-f32 (2 KB bank-aligned) offsets in PSUM
        wps = wpsum.tile([128, 1024], F32, tag="wps")
        # W[p, s*512 + (r, l)] = sum_k u^T[k, p] qflat[k, (r,l)]
        for sgl in range(ng):
            g = g0 + sgl
            nc.tensor.matmul(
                out=wps[:, sgl * 512: sgl * 512 + R * D],
                lhsT=utq[:D, g * 128: g * 128 + 128],
                rhs=qsb[:D, : R * D],
                start=True, stop=True,
            )
        wsb = wpool.tile([128, 640], BF16, tag="wsb")
        nc.scalar.copy(
            out=wsb[:, : ng * R * D].rearrange("p (s y) -> p s y", y=R * D),
            in_=wps[:].rearrange("p (s x) -> p s x", x=512)[:, :ng, : R * D])
        prod = wpool.tile([128, 640], BF16, tag="prod")
        w_ap = wsb[:, : ng * R * D].rearrange("p (s r l) -> p s r l", r=R, l=D)
        m_ap = (mt[:, g0: g0 + ng, :D][:, :, None, :]
                .to_broadcast([128, ng, R, D]))
        nc.vector.tensor_tensor(
            out=prod[:, : ng * R * D].rearrange("p (s r l) -> p s r l", r=R, l=D),
            in0=w_ap, in1=m_ap, op=mybir.AluOpType.mult)
        lgo = j * GROUPS * R
        nc.vector.tensor_reduce(
            out=logits[:, lgo + g0 * R: lgo + (g0 + ng) * R]
            .rearrange("p (s r) -> p s r", r=R),
            in_=prod[:, : ng * R * D].rearrange("p (s r l) -> p s r l", r=R, l=D),
            axis=mybir.AxisListType.X, op=mybir.AluOpType.add)
    return gather_i


def emit_softmax(nc, s, t0, logits, out, wpool):
    # ---- batched log_softmax over r for SUPER tiles at once ----
    k = SUPER * GROUPS
    mx = wpool.tile([128, k], F32, tag="mx")
    sm = wpool.tile([128, k], F32, tag="sm")
    ls = wpool.tile([128, k], F32, tag="ls")
    xm = wpool.tile([128, k * R], F32, tag="xm")
    ex = wpool.tile([128, k * R], F32, tag="ex")
    res = wpool.tile([128, k * R], F32, tag="res")
    lg3 = logits[:].rearrange("p (k r) -> p k r", r=R)
    nc.vector.tensor_reduce(
        out=mx[:], in_=lg3, axis=mybir.AxisListType.X, op=mybir.AluOpType.max)
    nc.vector.tensor_tensor(
        out=xm[:].rearrange("p (k r) -> p k r", r=R),
        in0=lg3, in1=mx[:][:, :, None].to_broadcast([128, k, R]),
        op=mybir.AluOpType.subtract)
    nc.scalar.activation(out=ex[:], in_=xm[:], func=mybir.ActivationFunctionType.Exp)
    nc.vector.tensor_reduce(
        out=sm[:], in_=ex[:].rearrange("p (k r) -> p k r", r=R),
        axis=mybir.AxisListType.X, op=mybir.AluOpType.add)
    nc.scalar.activation(out=ls[:], in_=sm[:], func=mybir.ActivationFunctionType.Ln)
    nc.vector.tensor_tensor(
        out=res[:].rearrange("p (k r) -> p k r", r=R),
        in0=xm[:].rearrange("p (k r) -> p k r", r=R),
        in1=ls[:][:, :, None].to_broadcast([128, k, R]),
        op=mybir.AluOpType.subtract)
    # edge (s, t, g, p) -> out row s*CAP + t*TILE_E + g*128 + p
    base = s * CAP + t0 * TILE_E
    nc.sync.dma_start(
        out=out[base: base + SUPER * TILE_E, :].rearrange("(g p) r -> p g r", p=128),
        in_=res[:].rearrange("p (g r) -> p g r", r=R))


_NC_CACHE = {}


def _get_nc():
    if "nc" not in _NC_CACHE:
        nc = build_nc()
        nc.finalize()
        _NC_CACHE["nc"] = nc
    return _NC_CACHE["nc"]


def _wrap16(a):
    """[n] int16 -> [128, n/16]: idx i at partition i%16, col i//16, x8 blocks."""
    w = a.reshape(-1, 16).T  # [16, n/16]
    return np.ascontiguousarray(np.tile(w, (8, 1)))


def kernel(z_user, z_movie, edge_label_index, Q):
    z_user = np.asarray(z_user, dtype=np.float32)
    z_movie = np.asarray(z_movie, dtype=np.float32)
    Q = np.asarray(Q, dtype=np.float32)
    eli = np.asarray(edge_label_index)
    row = eli[0].astype(np.int64)
    col = eli[1].astype(np.int64)
    e_total = row.shape[0]

    # bf16 tables padded to 256B rows
    zu_b = np.zeros((N_ROWS, 2 * D), dtype=ml_dtypes.bfloat16)
    zu_b[:, :D] = z_user.astype(ml_dtypes.bfloat16)
    zm_b = np.zeros((N_ROWS, 2 * D), dtype=ml_dtypes.bfloat16)
    zm_b[:, :D] = z_movie.astype(ml_dtypes.bfloat16)

    # block-diag [[Qflat, 0], [0, Qflat]] bf16, Qflat[k, (r,l)] = Q[r,k,l]
    qflat = np.transpose(Q, (1, 0, 2)).reshape(D, R * D)
    qbd = np.zeros((128, 2 * R * D), dtype=ml_dtypes.bfloat16)
    qbd[:D, : R * D] = qflat.astype(ml_dtypes.bfloat16)
    qbd[D:, R * D:] = qflat.astype(ml_dtypes.bfloat16)

    # 4x4 cell assignment
    cell = (row // CHUNK) * GRID + (col // CHUNK)
    order = np.argsort(cell, kind="stable")
    cell_sorted = cell[order]
    counts = np.bincount(cell_sorted, minlength=GRID * GRID)
    assert counts.max() <= CAP, f"cell overflow: {counts.max()} > {CAP}"
    starts = np.zeros(GRID * GRID + 1, dtype=np.int64)
    np.cumsum(counts, out=starts[1:])

    nc = _get_nc()

    in_maps = []
    gathers = []  # (core, slot, edge_ids) for unshard
    for c in range(N_CORES):
        a = c // 2
        bs = (2 * (c % 2), 2 * (c % 2) + 1)
        m = {
            "zu": np.ascontiguousarray(zu_b[a * CHUNK:(a + 1) * CHUNK]),
            "qbd": qbd,
        }
        for s, b in enumerate(bs):
            cid = a * GRID + b
            ids = order[starts[cid]:starts[cid + 1]]
            n = len(ids)
            iu = np.zeros(CAP, dtype=np.int16)
            im = np.zeros(CAP, dtype=np.int16)
            iu[:n] = (row[ids] - a * CHUNK).astype(np.int16)
            im[:n] = (col[ids] - b * CHUNK).astype(np.int16)
            m[f"zm{s}"] = np.ascontiguousarray(zm_b[b * CHUNK:(b + 1) * CHUNK])
            m[f"idxu{s}"] = _wrap16(iu)
            m[f"idxm{s}"] = _wrap16(im)
            gathers.append((c, s, ids))
        in_maps.append(m)

    trace = bool(int(os.environ.get("BK_TRACE", "0"))) and _ensure_ntff_hook()
    res = run_bass_kernel_spmd(nc, in_maps, list(range(N_CORES)), trace=trace)
    if trace:
        kernel.last_exec_time_ns = res.exec_time_ns
        kernel.last_mean_exec_time_ns = res.mean_exec_time_ns
        kernel.last_results = res

    out_full = np.empty((e_total, R), dtype=np.float32)
    for c, s, ids in gathers:
        rows_c = res.results[c]["out"][s * CAP: s * CAP + len(ids)]
        out_full[ids] = rows_c
    return out_full


def _ensure_ntff_hook():
    """Register the axon NTFF profiling hook if the container didn't."""
    import sys
    import types

    try:
        from antenv.axon_hooks import get_axon_ntff_profile_hook  # noqa: F401

        return True
    except ImportError:
        pass
    try:
        from trn_agent_boot.trn_boot import _ntff_profile_via_ctypes

        hook = _ntff_profile_via_ctypes("/opt/axon/libaxon_pjrt.so")
    except Exception as e:
        print("ntff hook unavailable:", e)
        return False
    if hook is None:
        print("ntff hook unavailable: old libaxon_pjrt.so")
        return False
    mod = types.ModuleType("antenv.axon_hooks")
    state = {"hook": hook}
    mod.get_axon_ntff_profile_hook = lambda: state["hook"]
    mod.set_axon_ntff_profile_hook = lambda h: state.__setitem__("hook", h)
    sys.modules["antenv.axon_hooks"] = mod
    import antenv

    antenv.axon_hooks = mod
    return True


# revision 22
# speedup vs baseline: 1.0682x; 1.0682x over previous
"""Trainium2 Bass kernel for a bilinear decoder:

    u = z_user[row]; m = z_movie[col]                      # [E, 64] gathers
    logits[e, r] = u_e^T Q_r m_e                           # [E, 5]
    out = log_softmax(logits, axis=-1)

Strategy (v2): 2-D cell sharding + batched SWDGE dma_gather.

The SWDGE batched-gather ucode (InstDMAGatherAnt, mlp GPSIMD library,
compiled via Bacc which auto-inserts the library load) gathers up to
~1920 rows per instruction but its int16 indices address at most 32768
table rows.  So edges are sharded host-side into a 4x4 grid of
(user-chunk, movie-chunk) cells of 25000 rows each -- two cells per
core, sharing one z_user slice -- and every gather addresses a <=25000
row slice with int16 indices.  Tables are converted host-side to bf16
padded to 256 B rows (the gather's granularity).

Per 896-edge tile: two 896-index dma_gathers (round-robin over 4 SWDGE
queues; 57 descriptors each, two in flight per 128-descriptor ring), 7
per-group PE transposes (bf16 PSUM), one ACT copy to SBUF, 7 matmuls
(u^T chunks x Qflat bf16, outputs at 2 KB bank-aligned PSUM offsets --
matmul outputs that straddle a PSUM bank boundary silently corrupt),
ACT f32->bf16 W copy, bf16 DVE multiply+reduce.  log_softmax is
batched over 4-tile super-tiles so the ACT engine's Exp/Ln activation
table reloads (~1.3 us each) amortize; that batching took 1.22 ms ->
0.80 ms.

Empirical limits (measured here): >1024 indices per gather overflows
the 128-descriptor/lane SWDGE ring and wedges the core (the ring size
is an NRT-side constant; neither dynamic_dma_scratch_size nor walrus'
--dynamic-dma-scratch-size-per-partition moves it).  The Pool engine
serializes desc-gen at ~994 ns + ~1.2 ns/idx per instruction -- with
288 gathers/core that is ~620 us and is this kernel's wall.

Measured on 8 axon trn2 cores: rel err 3.4e-3 (bf16 tables/compute;
harness gate 2e-2), HW exec time 0.80 ms vs 2.76 ms baseline (3.45x).
"""

import os
import numpy as np
import ml_dtypes

import concourse.bacc as bacc
import concourse.bass as bass
import concourse.mybir as mybir
import concourse.tile as tile
from concourse.masks import make_identity
from concourse.bass_utils import run_bass_kernel_spmd

F32 = mybir.dt.float32
BF16 = mybir.dt.bfloat16
I16 = mybir.dt.int16

N_ROWS = 100000
E_TOTAL = 1000000
D = 64
R = 5
N_CORES = 8

CHUNK = 25000            # table rows per cell slice (fits int16 indices)
GRID = 4                 # 4x4 grid of (user-chunk, movie-chunk) cells
CAP = 64512              # padded edges per cell; 63 tiles of 1024
TILE_E = 1024            # edges per gather instruction (65 descs/ring)
NT_CELL = CAP // TILE_E  # 63
GROUPS = TILE_E // 128   # 8 edge groups per tile
N_QUEUES = 4
SUPER = 7             # tiles per softmax flush (amortizes ACT table reloads)


def build_nc():
    nc = bacc.Bacc("TRN2", num_swdge_queues=N_QUEUES)
    zu = nc.dram_tensor("zu", [CHUNK, 2 * D], BF16, kind="ExternalInput")
    zm0 = nc.dram_tensor("zm0", [CHUNK, 2 * D], BF16, kind="ExternalInput")
    zm1 = nc.dram_tensor("zm1", [CHUNK, 2 * D], BF16, kind="ExternalInput")
    # int16 idx streams, 16-partition wrapped and replicated to 128 partitions
    idxu = [nc.dram_tensor(f"idxu{s}", [128, CAP // 16], I16, kind="ExternalInput")
            for s in range(2)]
    idxm = [nc.dram_tensor(f"idxm{s}", [128, CAP // 16], I16, kind="ExternalInput")
            for s in range(2)]
    # block-diag [[Qflat, 0], [0, Qflat]] with Qflat[k, (r, l)] = Q[r, k, l]
    qbd = nc.dram_tensor("qbd", [128, 2 * R * D], BF16, kind="ExternalInput")
    out = nc.dram_tensor("out", [2 * CAP, R], F32, kind="ExternalOutput")

    gather_i = 0
    with tile.TileContext(nc) as tc:
        with (
            tc.tile_pool(name="const", bufs=1) as cpool,
            tc.tile_pool(name="io", bufs=8) as iopool,
            tc.tile_pool(name="work", bufs=3) as wpool,
            tc.tile_pool(name="ps_t", bufs=2, space="PSUM") as tpsum,
            tc.tile_pool(name="ps_w", bufs=3, space="PSUM") as wpsum,
        ):
            ident = cpool.tile([128, 128], BF16)
            make_identity(nc, ident[:])
            qsb = cpool.tile([128, 2 * R * D], BF16)
            nc.sync.dma_start(out=qsb[:], in_=qbd[:])
            idxu_sb = [cpool.tile([128, CAP // 16], I16, name=f"idxu_sb{i}") for i in range(2)]
            idxm_sb = [cpool.tile([128, CAP // 16], I16, name=f"idxm_sb{i}") for i in range(2)]
            for s in range(2):
                nc.sync.dma_start(out=idxu_sb[s][:], in_=idxu[s][:])
                nc.sync.dma_start(out=idxm_sb[s][:], in_=idxm[s][:])

            for s in range(2):
                zm = zm0 if s == 0 else zm1
                for t0 in range(0, NT_CELL, SUPER):
                    logits = wpool.tile([128, SUPER * GROUPS * R], F32, tag="lg")
                    for j in range(SUPER):
                        gather_i = emit_tile(
                            nc, s, t0 + j, j, zu, zm, idxu_sb[s], idxm_sb[s],
                            qsb, ident, logits, iopool, wpool, tpsum, wpsum,
                            gather_i)
                    emit_softmax(nc, s, t0, logits, out, wpool)
    return nc


def emit_tile(nc, s, t, j, zu, zm, idxu_sb, idxm_sb, qsb, ident, logits,
              iopool, wpool, tpsum, wpsum, gather_i):
    c0 = t * (TILE_E // 16)
    c1 = (t + 1) * (TILE_E // 16)

    # ---- gathers: [128, g, 128] bf16; idx i -> partition i%128, group i//128
    ut = iopool.tile([128, GROUPS, 128], BF16, tag="ut")
    mt = iopool.tile([128, GROUPS, 128], BF16, tag="mt")
    nc.gpsimd.dma_gather(
        out_ap=ut[:], in_ap=zu[:], idxs_ap=idxu_sb[:, c0:c1],
        num_idxs=TILE_E, num_idxs_reg=TILE_E, elem_size=2 * D,
        queue_num=gather_i % N_QUEUES)
    gather_i += 1
    nc.gpsimd.dma_gather(
        out_ap=mt[:], in_ap=zm[:], idxs_ap=idxm_sb[:, c0:c1],
        num_idxs=TILE_E, num_idxs_reg=TILE_E, elem_size=2 * D,
        queue_num=gather_i % N_QUEUES)
    gather_i += 1

    # ---- transposes: u^T per group, [64 l-partitions, 128 edge cols]
    n_pairs = (GROUPS + 1) // 2  # 4 (last pair is a single group)
    tps = tpsum.tile([128, GROUPS * 128], BF16, tag="tps")
    for g in range(GROUPS):
        nc.tensor.transpose(
            out=tps[:D, g * 128: g * 128 + 128],
            in_=ut[:, g, :D],
            identity=ident[:],
        )
    utq = wpool.tile([128, GROUPS * 128], BF16, tag="utq")
    nc.scalar.copy(out=utq[:D], in_=tps[:D])

    for h in range(n_pairs):
        g0 = 2 * h
        ng = min(2, GROUPS - g0)
        # W chunks at 512-f32 (2 KB bank-aligned) offsets in PSUM
        wps = wpsum.tile([128, 1024], F32, tag="wps")
        # W[p, s*512 + (r, l)] = sum_k u^T[k, p] qflat[k, (r,l)]
        for sgl in range(ng):
            g = g0 + sgl
            nc.tensor.matmul(
                out=wps[:, sgl * 512: sgl * 512 + R * D],
                lhsT=utq[:D, g * 128: g * 128 + 128],
                rhs=qsb[:D, : R * D],
                start=True, stop=True,
            )
        wsb = wpool.tile([128, 640], BF16, tag="wsb")
        nc.scalar.copy(
            out=wsb[:, : ng * R * D].rearrange("p (s y) -> p s y", y=R * D),
            in_=wps[:].rearrange("p (s x) -> p s x", x=512)[:, :ng, : R * D])
        prod = wpool.tile([128, 640], BF16, tag="prod")
        w_ap = wsb[:, : ng * R * D].rearrange("p (s r l) -> p s r l", r=R, l=D)
        m_ap = (mt[:, g0: g0 + ng, :D][:, :, None, :]
                .to_broadcast([128, ng, R, D]))
        nc.vector.tensor_tensor(
            out=prod[:, : ng * R * D].rearrange("p (s r l) -> p s r l", r=R, l=D),
            in0=w_ap, in1=m_ap, op=mybir.AluOpType.mult)
        lgo = j * GROUPS * R
        nc.vector.tensor_reduce(
            out=logits[:, lgo + g0 * R: lgo + (g0 + ng) * R]
            .rearrange("p (s r) -> p s r", r=R),
            in_=prod[:, : ng * R * D].rearrange("p (s r l) -> p s r l", r=R, l=D),
            axis=mybir.AxisListType.X, op=mybir.AluOpType.add)
    return gather_i


def emit_softmax(nc, s, t0, logits, out, wpool):
    # ---- batched log_softmax over r for SUPER tiles at once ----
    k = SUPER * GROUPS
    mx = wpool.tile([128, k], F32, tag="mx")
    sm = wpool.tile([128, k], F32, tag="sm")
    ls = wpool.tile([128, k], F32, tag="ls")
    xm = wpool.tile([128, k * R], F32, tag="xm")
    ex = wpool.tile([128, k * R], F32, tag="ex")
    res = wpool.tile([128, k * R], F32, tag="res")
    lg3 = logits[:].rearrange("p (k r) -> p k r", r=R)
    nc.vector.tensor_reduce(
        out=mx[:], in_=lg3, axis=mybir.AxisListType.X, op=mybir.AluOpType.max)
    nc.vector.tensor_tensor(
        out=xm[:].rearrange("p (k r) -> p k r", r=R),
        in0=lg3, in1=mx[:][:, :, None].to_broadcast([128, k, R]),
        op=mybir.AluOpType.subtract)
    nc.scalar.activation(out=ex[:], in_=xm[:], func=mybir.ActivationFunctionType.Exp)
    nc.vector.tensor_reduce(
        out=sm[:], in_=ex[:].rearrange("p (k r) -> p k r", r=R),
        axis=mybir.AxisListType.X, op=mybir.AluOpType.add)
    nc.scalar.activation(out=ls[:], in_=sm[:], func=mybir.ActivationFunctionType.Ln)
    nc.vector.tensor_tensor(
        out=res[:].rearrange("p (k r) -> p k r", r=R),
        in0=xm[:].rearrange("p (k r) -> p k r", r=R),
        in1=ls[:][:, :, None].to_broadcast([128, k, R]),
        op=mybir.AluOpType.subtract)
    # edge (s, t, g, p) -> out row s*CAP + t*TILE_E + g*128 + p
    base = s * CAP + t0 * TILE_E
    nc.sync.dma_start(
        out=out[base: base + SUPER * TILE_E, :].rearrange("(g p) r -> p g r", p=128),
        in_=res[:].rearrange("p (g r) -> p g r", r=R))


_NC_CACHE = {}


def _get_nc():
    if "nc" not in _NC_CACHE:
        nc = build_nc()
        nc.finalize()
        _NC_CACHE["nc"] = nc
    return _NC_CACHE["nc"]


def _wrap16(a):
    """[n] int16 -> [128, n/16]: idx i at partition i%16, col i//16, x8 blocks."""
    w = a.reshape(-1, 16).T  # [16, n/16]
    return np.ascontiguousarray(np.tile(w, (8, 1)))


def kernel(z_user, z_movie, edge_label_index, Q):
    z_user = np.asarray(z_user, dtype=np.float32)
    z_movie = np.asarray(z_movie, dtype=np.float32)
    Q = np.asarray(Q, dtype=np.float32)
    eli = np.asarray(edge_label_index)
    row = eli[0].astype(np.int64)
    col = eli[1].astype(np.int64)
    e_total = row.shape[0]

    # bf16 tables padded to 256B rows
    zu_b = np.zeros((N_ROWS, 2 * D), dtype=ml_dtypes.bfloat16)
    zu_b[:, :D] = z_user.astype(ml_dtypes.bfloat16)
    zm_b = np.zeros((N_ROWS, 2 * D), dtype=ml_dtypes.bfloat16)
    zm_b[:, :D] = z_movie.astype(ml_dtypes.bfloat16)

    # block-diag [[Qflat, 0], [0, Qflat]] bf16, Qflat[k, (r,l)] = Q[r,k,l]
    qflat = np.transpose(Q, (1, 0, 2)).reshape(D, R * D)
    qbd = np.zeros((128, 2 * R * D), dtype=ml_dtypes.bfloat16)
    qbd[:D, : R * D] = qflat.astype(ml_dtypes.bfloat16)
    qbd[D:, R * D:] = qflat.astype(ml_dtypes.bfloat16)

    # 4x4 cell assignment
    cell = (row // CHUNK) * GRID + (col // CHUNK)
    order = np.argsort(cell, kind="stable")
    cell_sorted = cell[order]
    counts = np.bincount(cell_sorted, minlength=GRID * GRID)
    assert counts.max() <= CAP, f"cell overflow: {counts.max()} > {CAP}"
    starts = np.zeros(GRID * GRID + 1, dtype=np.int64)
    np.cumsum(counts, out=starts[1:])

    nc = _get_nc()

    in_maps = []
    gathers = []  # (core, slot, edge_ids) for unshard
    for c in range(N_CORES):
        a = c // 2
        bs = (2 * (c % 2), 2 * (c % 2) + 1)
        m = {
            "zu": np.ascontiguousarray(zu_b[a * CHUNK:(a + 1) * CHUNK]),
            "qbd": qbd,
        }
        for s, b in enumerate(bs):
            cid = a * GRID + b
            ids = order[starts[cid]:starts[cid + 1]]
            n = len(ids)
            iu = np.zeros(CAP, dtype=np.int16)
            im = np.zeros(CAP, dtype=np.int16)
            iu[:n] = (row[ids] - a * CHUNK).astype(np.int16)
            im[:n] = (col[ids] - b * CHUNK).astype(np.int16)
            m[f"zm{s}"] = np.ascontiguousarray(zm_b[b * CHUNK:(b + 1) * CHUNK])
            m[f"idxu{s}"] = _wrap16(iu)
            m[f"idxm{s}"] = _wrap16(im)
            gathers.append((c, s, ids))
        in_maps.append(m)

    trace = bool(int(os.environ.get("BK_TRACE", "0"))) and _ensure_ntff_hook()
    res = run_bass_kernel_spmd(nc, in_maps, list(range(N_CORES)), trace=trace)
    if trace:
        kernel.last_exec_time_ns = res.exec_time_ns
        kernel.last_mean_exec_time_ns = res.mean_exec_time_ns
        kernel.last_results = res

    out_full = np.empty((e_total, R), dtype=np.float32)
    for c, s, ids in gathers:
        rows_c = res.results[c]["out"][s * CAP: s * CAP + len(ids)]
        out_full[ids] = rows_c
    return out_full


def _ensure_ntff_hook():
    """Register the axon NTFF profiling hook if the container didn't."""
    import sys
    import types

    try:
        from antenv.axon_hooks import get_axon_ntff_profile_hook  # noqa: F401

        return True
    except ImportError:
        pass
    try:
        from trn_agent_boot.trn_boot import _ntff_profile_via_ctypes

        hook = _ntff_profile_via_ctypes("/opt/axon/libaxon_pjrt.so")
    except Exception as e:
        print("ntff hook unavailable:", e)
        return False
    if hook is None:
        print("ntff hook unavailable: old libaxon_pjrt.so")
        return False
    mod = types.ModuleType("antenv.axon_hooks")
    state = {"hook": hook}
    mod.get_axon_ntff_profile_hook = lambda: state["hook"]
    mod.set_axon_ntff_profile_hook = lambda h: state.__setitem__("hook", h)
    sys.modules["antenv.axon_hooks"] = mod
    import antenv

    antenv.axon_hooks = mod
    return True
